# revision 15
# baseline (speedup 1.0000x reference)
"""DeBut 2D-conv kernel for Trainium2 (8 NeuronCores, data-parallel over batch).

Math: the reference is im2col(x) -> chain of 3 deformable-butterfly factors
-> +bias -> reshape.  The three factors compose into a single block-diagonal
matrix M (256x1152): M[o, i] != 0 only for i in [18*(o//4), 18*(o//4)+18).
With im2col feature order (kh, kw, c), feature chunk kk*128..kk*128+128 of a
pixel (h, w) is just x[:, h+kh-1, w+kw-1] -- a spatially shifted channel
vector.  So conv == 9 shifted [128 x 128] matmuls accumulated in PSUM, and
because of the band structure each 128-feature chunk only touches one or two
128-output-channel halves: 10 (chunk, half) pairs have nonzero weights.
10 matmuls/tile is PE-optimal for this structure: 9 needs a third PSUM bank
whose evacuation slices cost more engine time than the saved matmul.

Per core: 2 images; x is zero-padded to 58x58 on host (so shifts are exact
strided views of one SBUF tile) and cast to bf16; weights composed on host in
float64 and cast to bf16; accumulation is fp32 in PSUM.  DMA is chunked fine
(per-tap weight loads, ~8-row x loads, 2/2/2/1-tile stores) and split across
the two HWDGE queues (SP + ACT) so the PE stream is never gated on one big
transfer and HBM traffic stays smooth across the 8 phase-aligned cores.
"""

import numpy as np
import ml_dtypes

import concourse.bass as bass
import concourse.tile as tile
from concourse import bacc, mybir
from concourse.bass_utils import run_bass_kernel_spmd

# Problem constants (hardcoded; kernel.py must be self-contained).
B, C_IN, H, W = 16, 128, 56, 56
C_OUT = 256
HP, WP = H + 2, W + 2  # zero-padded spatial dims (58, 58)
N_CORES = 8
B_CORE = B // N_CORES  # 2 images per core
R_SHAPES = [[512, 1152, 4, 9, 1], [512, 512, 4, 4, 1], [256, 512, 2, 4, 2]]

ROWS_PER_TILE = 8            # 8 rows x 56 cols = 448 pixels per PSUM tile
FREE = ROWS_PER_TILE * W     # 448 <= 512 fp32 per PSUM bank
# (row0, nrows) pixel tiles: six 8-row tiles + two 4-row tiles.  Total PE
# cycles are identical (matmul cost ~ free size), but the final tile being
# half-size shortens every data-dependent link of the kernel tail chain
# (last matmul -> evac -> descriptor gen -> DMA -> sem) by ~0.8us.
TILES = [(0, 8), (8, 8), (16, 8), (24, 8), (32, 8), (40, 8), (48, 4), (52, 4)]
NT = len(TILES)

# (m, kk) pairs with a nonzero weight band: m = output-channel half (0/1),
# kk = kh*3+kw 3x3 tap index.  Feature chunk kk covers im2col features
# [128kk, 128kk+128) -> blocks k3 in ~[7.1kk, 7.1kk+7.1] -> channels 4*k3.
PAIRS = [(0, 0), (0, 1), (0, 2), (0, 3), (0, 4),
         (1, 4), (1, 5), (1, 6), (1, 7), (1, 8)]
KKS = {0: [0, 1, 2, 3, 4], 1: [4, 5, 6, 7, 8]}
JIDX = {pair: j for j, pair in enumerate(PAIRS)}

# every matmul writes the full 128-partition output chunk (band weights
# zero-padded to M=128): simplest exact PSUM accumulate/overwrite semantics
WIN = {pair: (0, 128) for pair in PAIRS}
WCOL = {pair: JIDX[pair] * 128 for pair in PAIRS}
W_COLS = len(PAIRS) * 128  # 1280

BF16 = mybir.dt.bfloat16
F32 = mybir.dt.float32

_CACHE = {}


def _debut_matrix(twiddle: np.ndarray) -> np.ndarray:
    """Compose the butterfly chain into M (256x1152) with out = M @ x."""
    out = np.eye(1152, dtype=np.float64)
    p = 0
    for (out_size, in_size, row, col, diag) in R_SHAPES:
        num_p = col * out_size
        blocks = in_size // (col * diag)
        t = (twiddle[p:p + num_p].astype(np.float64)
             .reshape(blocks, diag, row, col).transpose(0, 2, 3, 1))
        xr = out.reshape(-1, blocks, col, diag)
        out = np.einsum('krcd,nkcd->nkrd', t, xr).reshape(-1, out_size)
        p += num_p
    return out.T  # (256, 1152)


def _build_nc(repeat: int = 1, probe: str = "") -> bacc.Bacc:
    """repeat > 1 wraps the whole compute body in a device-side For_i loop
    (used only by the timing harness; the graded path uses repeat=1).
    probe='peonly' strips DMA/evacuation to measure the pure matmul stream."""
    nc = bacc.Bacc("TRN2", target_bir_lowering=False, debug=False,
                   num_devices=N_CORES)
    xd = nc.dram_tensor("xpad", [B_CORE, C_IN, HP, WP], BF16,
                        kind="ExternalInput")
    wd = nc.dram_tensor("wmat", [C_IN, W_COLS], BF16,
                        kind="ExternalInput")
    bd = nc.dram_tensor("bias2", [128, 2], F32, kind="ExternalInput")
    yd = nc.dram_tensor("y", [B_CORE, C_OUT, H, W], BF16,
                        kind="ExternalOutput")

    with tile.TileContext(nc) as tc:
        with (
            tc.tile_pool(name="wpool", bufs=1) as wpool,
            tc.tile_pool(name="bpool", bufs=1) as bpool,
            tc.tile_pool(name="xpool", bufs=3) as xpool,
            tc.tile_pool(name="opool", bufs=6) as opool,
            tc.tile_pool(name="psum", bufs=8, space="PSUM") as ppool,
        ):
            # HWDGE descriptor generation costs ~632ns + ~3.4ns/descriptor
            # (128 descriptors per DMA here, one per partition), so FEW BIG
            # DMAs win.  Weight split at pair 5 (col 640) is the no-gap
            # point: chunk 1 arrives with x0 and covers the tile-0 half-0
            # stream, chunk 2 lands before the stream reaches pair 5.  bias
            # loads once, after the weights (first evacuation has PSUM-
            # buffer slack to wait).  Queues: all x loads on SP, all stores
            # on ACT — a store dma_start on ACT would stall ACT's next
            # evacuation if its data weren't ready yet (sequencer head-of-
            # line), so store chunks are issued one tile late (see body).
            w_split = 640
            w_t = wpool.tile([C_IN, W_COLS], BF16)
            nc.scalar.dma_start(w_t[:, :w_split], wd.ap()[:, :w_split])
            nc.scalar.dma_start(w_t[:, w_split:], wd.ap()[:, w_split:])
            bias_t = bpool.tile([128, 2], F32)
            nc.scalar.dma_start(bias_t[:], bd.ap()[:])

            # x-load row chunks (padded rows incl. halo): first chunk covers
            # just pixel-tile 0 so the matmul stream starts ~0.5us in
            X_CHUNKS = [(0, 12), (12, 34), (34, HP)]
            # Store chunks are issued one tile after their data completes
            # (at the next tile's m=0 evacuation) so the ACT sequencer's
            # store dma_starts never wait on a pending DVE evacuation and
            # stall ACT's own next evacuation (head-of-line).  Tail: half-0
            # rows 48..56 go out as one early store during the half-1
            # matmuls; the only post-loop DMA is the tiny 4-row half-1
            # store, keeping the last-matmul -> kernel-end chain short.
            STORE_CHUNKS = {4: [(0, 0, 32), (1, 0, 32)],
                            6: [(0, 32, 48), (1, 32, 48)],
                            7: [(0, 48, 56), (1, 48, 52)]}
            LAST_CHUNK = [(1, 52, 56)]

            def load_x(b):
                xp_t = xpool.tile([C_IN, HP, WP], BF16, name=f"xp_{b}",
                                  tag="xp")
                for r0, r1 in X_CHUNKS:
                    nc.sync.dma_start(xp_t[:, r0:r1, :],
                                      xd.ap()[b, :, r0:r1, :])
                return xp_t

            def store(b, o_img, chunks):
                for m, s0, s1 in chunks:
                    nc.scalar.dma_start(
                        yd.ap()[b, m * 128:(m + 1) * 128, s0:s1, :],
                        o_img[m][:, s0 * W:s1 * W])

            def body():
                for b in range(B_CORE):
                    xp_t = load_x(b)
                    o_img = {}
                    if probe != "peonly":
                        for m in range(2):
                            o_img[m] = opool.tile([128, H * W], BF16,
                                                  name=f"o_img_{b}_{m}",
                                                  tag="o_img")
                    for t, (r0, nr) in enumerate(TILES):
                        for m in range(2):
                            ps = ppool.tile([128, nr * W], F32)
                            kks = KKS[m]
                            for i, kk in enumerate(kks):
                                kh, kw = divmod(kk, 3)
                                base, msize = WIN[(m, kk)]
                                col = WCOL[(m, kk)]
                                rhs = xp_t[:, r0 + kh: r0 + kh + nr,
                                           kw: kw + W]
                                nc.tensor.matmul(
                                    ps[base:base + msize, :],
                                    w_t[:, col:col + msize], rhs,
                                    start=(i == 0), stop=(i == len(kks) - 1))
                            if probe == "peonly":
                                continue
                            # split PSUM evacuation across ACT and DVE; whole
                            # image-half accumulates in SBUF so stores are a
                            # few big descriptors per partition
                            osl = o_img[m][:, r0 * W:(r0 + nr) * W]
                            if m == 0:
                                nc.scalar.add(osl, ps[:], bias_t[:, m:m + 1])
                                if t in STORE_CHUNKS:
                                    store(b, o_img, STORE_CHUNKS[t])
                            else:
                                nc.vector.tensor_scalar_add(
                                    osl, ps[:], bias_t[:, m:m + 1])
                    if probe != "peonly":
                        store(b, o_img, LAST_CHUNK)

            # Warmup matmuls on a scratch tile during the DMA-load head: the
            # PE HAM activity window starts seeing a busy PE at t~0, so the
            # 1.2->2.4 GHz un-throttle fires ~1-2us earlier than if the first
            # real matmul (gated on the x DMA) started the clock.  A few
            # large-free warmups extend continuous PE busy across the ~2.5us
            # DMA head so the ramp never resets before the real stream.
            # split memset so the first (64-wide) warmups aren't gated on
            # zeroing the whole 448-wide scratch: DVE clears the first 64
            # cols (~70ns), GPSIMD the rest in parallel
            wm_src = wpool.tile([C_IN, 448], BF16, name="wm_src")
            nc.vector.memset(wm_src[:, :64], 0.0)
            nc.gpsimd.memset(wm_src[:, 64:], 0.0)
            wm_ps = ppool.tile([64, FREE], F32, name="wm_ps", tag="ps")
            for _ in range(6):
                nc.tensor.matmul(wm_ps[:, :64], wm_src[:, :64],
                                 wm_src[:, :64], start=True, stop=True)
            for _ in range(6):
                nc.tensor.matmul(wm_ps[:], wm_src[:, :64], wm_src[:],
                                 start=True, stop=True)

            if repeat == 1:
                body()
            else:
                with tc.For_i(0, repeat, 1,
                              hint_engines=(mybir.EngineType.PE,
                                            mybir.EngineType.Activation,
                                            mybir.EngineType.SP)):
                    body()
    nc.finalize()
    return nc


def _prep_inputs(x: np.ndarray, twiddle: np.ndarray, bias: np.ndarray):
    """Host-side: pad + cast x, compose weights, arrange per-core in_maps."""
    x = np.asarray(x, dtype=np.float32)
    xpad = np.zeros((B, C_IN, HP, WP), dtype=ml_dtypes.bfloat16)
    xpad[:, :, 1:1 + H, 1:1 + W] = x.astype(ml_dtypes.bfloat16)

    M = _debut_matrix(np.asarray(twiddle, dtype=np.float32))
    wmat = np.zeros((C_IN, W_COLS), dtype=np.float64)
    for (m, kk) in PAIRS:
        # lhsT layout: wmat[c, WCOL + i] = M[128m + base + i, 128kk + c]
        base, msize = WIN[(m, kk)]
        col = WCOL[(m, kk)]
        wmat[:, col:col + msize] = M[m * 128 + base:m * 128 + base + msize,
                                     kk * 128:(kk + 1) * 128].T
    wmat = wmat.astype(ml_dtypes.bfloat16)

    bias2 = np.asarray(bias, dtype=np.float32).reshape(2, 128).T.copy()

    in_maps = []
    for core in range(N_CORES):
        in_maps.append({
            "xpad": xpad[core * B_CORE:(core + 1) * B_CORE],
            "wmat": wmat,
            "bias2": bias2,
        })
    return in_maps


def kernel(x: np.ndarray, twiddle: np.ndarray, bias: np.ndarray) -> np.ndarray:
    if "nc" not in _CACHE:
        _CACHE["nc"] = _build_nc()
    nc = _CACHE["nc"]
    in_maps = _prep_inputs(x, twiddle, bias)
    res = run_bass_kernel_spmd(nc, in_maps, list(range(N_CORES)))
    out = np.concatenate(
        [np.asarray(res.results[i]["y"]) for i in range(N_CORES)], axis=0)
    return np.ascontiguousarray(out.astype(np.float32))


# revision 19
# speedup vs baseline: 1.2772x; 1.2772x over previous
"""DeBut 2D-conv kernel for Trainium2 (8 NeuronCores, data-parallel over batch).

Math: the reference is im2col(x) -> chain of 3 deformable-butterfly factors
-> +bias -> reshape.  The three factors compose into a single block-diagonal
matrix M (256x1152): M[o, i] != 0 only for i in [18*(o//4), 18*(o//4)+18).
With im2col feature order (kh, kw, c), feature chunk kk*128..kk*128+128 of a
pixel (h, w) is just x[:, h+kh-1, w+kw-1] -- a spatially shifted channel
vector.  So conv == 9 shifted [128 x 128] matmuls accumulated in PSUM, and
because of the band structure each 128-feature chunk only touches one or two
128-output-channel halves: 10 (chunk, half) pairs have nonzero weights.
10 matmuls/tile is PE-optimal for this structure: 9 needs a third PSUM bank
whose evacuation slices cost more engine time than the saved matmul.

Per core: 2 images; x is zero-padded to 58x58 on host (so shifts are exact
strided views of one SBUF tile) and cast to bf16; weights composed on host in
float64 and cast to bf16; accumulation is fp32 in PSUM.  DMA is chunked fine
(per-tap weight loads, ~8-row x loads, 2/2/2/1-tile stores) and split across
the two HWDGE queues (SP + ACT) so the PE stream is never gated on one big
transfer and HBM traffic stays smooth across the 8 phase-aligned cores.
"""

import numpy as np
import ml_dtypes

import concourse.bass as bass
import concourse.tile as tile
from concourse import bacc, mybir
from concourse.bass_utils import run_bass_kernel_spmd

# Problem constants (hardcoded; kernel.py must be self-contained).
B, C_IN, H, W = 16, 128, 56, 56
C_OUT = 256
HP, WP = H + 2, W + 2  # zero-padded spatial dims (58, 58)
N_CORES = 8
B_CORE = B // N_CORES  # 2 images per core
R_SHAPES = [[512, 1152, 4, 9, 1], [512, 512, 4, 4, 1], [256, 512, 2, 4, 2]]

ROWS_PER_TILE = 8            # 8 rows x 56 cols = 448 pixels per PSUM tile
FREE = ROWS_PER_TILE * W     # 448 <= 512 fp32 per PSUM bank
# (row0, nrows) pixel tiles.  The LAST image ends with two 4-row tiles:
# total PE cycles are identical (matmul cost ~ free size), but a half-size
# final tile shortens every data-dependent link of the kernel tail chain
# (last matmul -> evac -> descriptor gen -> DMA -> sem) by ~0.8us.  Image 0
# is mid-kernel, so it keeps plain 8-row tiles (2 fewer evac ops).
TILES_8 = [(8 * t, 8) for t in range(7)]
TILES_TAIL = TILES_8[:6] + [(48, 4), (52, 4)]

# (m, kk) pairs with a nonzero weight band: m = output-channel half (0/1),
# kk = kh*3+kw 3x3 tap index.  Feature chunk kk covers im2col features
# [128kk, 128kk+128) -> blocks k3 in ~[7.1kk, 7.1kk+7.1] -> channels 4*k3.
PAIRS = [(0, 0), (0, 1), (0, 2), (0, 3), (0, 4),
         (1, 4), (1, 5), (1, 6), (1, 7), (1, 8)]
KKS = {0: [0, 1, 2, 3, 4], 1: [4, 5, 6, 7, 8]}
JIDX = {pair: j for j, pair in enumerate(PAIRS)}

# every matmul writes the full 128-partition output chunk (band weights
# zero-padded to M=128): simplest exact PSUM accumulate/overwrite semantics
WIN = {pair: (0, 128) for pair in PAIRS}
WCOL = {pair: JIDX[pair] * 128 for pair in PAIRS}
W_COLS = len(PAIRS) * 128  # 1280

BF16 = mybir.dt.bfloat16
F32 = mybir.dt.float32

_CACHE = {}


def _debut_matrix(twiddle: np.ndarray) -> np.ndarray:
    """Compose the butterfly chain into M (256x1152) with out = M @ x."""
    out = np.eye(1152, dtype=np.float64)
    p = 0
    for (out_size, in_size, row, col, diag) in R_SHAPES:
        num_p = col * out_size
        blocks = in_size // (col * diag)
        t = (twiddle[p:p + num_p].astype(np.float64)
             .reshape(blocks, diag, row, col).transpose(0, 2, 3, 1))
        xr = out.reshape(-1, blocks, col, diag)
        out = np.einsum('krcd,nkcd->nkrd', t, xr).reshape(-1, out_size)
        p += num_p
    return out.T  # (256, 1152)


def _build_nc(repeat: int = 1, probe: str = "") -> bacc.Bacc:
    """repeat > 1 wraps the whole compute body in a device-side For_i loop
    (used only by the timing harness; the graded path uses repeat=1).
    probe='peonly' strips DMA/evacuation to measure the pure matmul stream."""
    nc = bacc.Bacc("TRN2", target_bir_lowering=False, debug=False,
                   num_devices=N_CORES)
    xd = nc.dram_tensor("xpad", [B_CORE, C_IN, HP, WP], BF16,
                        kind="ExternalInput")
    wd = nc.dram_tensor("wmat", [C_IN, W_COLS], BF16,
                        kind="ExternalInput")
    bd = nc.dram_tensor("bias2", [128, 2], F32, kind="ExternalInput")
    yd = nc.dram_tensor("y", [B_CORE, C_OUT, H, W], BF16,
                        kind="ExternalOutput")

    with tile.TileContext(nc) as tc:
        with (
            tc.tile_pool(name="wpool", bufs=1) as wpool,
            tc.tile_pool(name="bpool", bufs=1) as bpool,
            tc.tile_pool(name="xpool", bufs=3) as xpool,
            tc.tile_pool(name="opool", bufs=6) as opool,
            tc.tile_pool(name="psum", bufs=8, space="PSUM") as ppool,
        ):
            # HWDGE descriptor generation costs ~632ns + ~3.4ns/descriptor
            # (128 descriptors per DMA here, one per partition), so FEW BIG
            # DMAs win.  Weight split at pair 5 (col 640) is the no-gap
            # point: chunk 1 arrives with x0 and covers the tile-0 half-0
            # stream, chunk 2 lands before the stream reaches pair 5.  bias
            # loads once, after the weights (first evacuation has PSUM-
            # buffer slack to wait).  Queues: all x loads on SP, all stores
            # on ACT — a store dma_start on ACT would stall ACT's next
            # evacuation if its data weren't ready yet (sequencer head-of-
            # line), so store chunks are issued one tile late (see body).
            w_split = 640
            w_t = wpool.tile([C_IN, W_COLS], BF16)
            nc.scalar.dma_start(w_t[:, :w_split], wd.ap()[:, :w_split])
            nc.scalar.dma_start(w_t[:, w_split:], wd.ap()[:, w_split:])
            bias_t = bpool.tile([128, 2], F32)
            nc.scalar.dma_start(bias_t[:], bd.ap()[:])

            # x-load row chunks (padded rows incl. halo): first chunk covers
            # just pixel-tile 0 so the matmul stream starts ~0.5us in
            X_CHUNKS = [(0, 12), (12, 34), (34, HP)]
            # Store chunks are issued one tile after their data completes
            # (at the next tile's m=0 evacuation) so the ACT sequencer's
            # store dma_starts never wait on a pending DVE evacuation and
            # stall ACT's own next evacuation (head-of-line).  Tail image:
            # half-0 rows 48..56 go out as one early store during the
            # half-1 matmuls; the only post-loop DMA is the tiny 4-row
            # half-1 store, keeping the last-matmul -> kernel-end chain
            # short.
            STORE_CHUNKS_8 = {4: [(0, 0, 32), (1, 0, 32)],
                              6: [(0, 32, 48), (1, 32, 48)]}
            LAST_CHUNK_8 = [(0, 48, 56), (1, 48, 56)]
            STORE_CHUNKS_TAIL = {4: [(0, 0, 32), (1, 0, 32)],
                                 6: [(0, 32, 48), (1, 32, 48)],
                                 7: [(0, 48, 56), (1, 48, 52)]}
            LAST_CHUNK_TAIL = [(1, 52, 56)]

            def load_x(b):
                xp_t = xpool.tile([C_IN, HP, WP], BF16, name=f"xp_{b}",
                                  tag="xp")
                for r0, r1 in X_CHUNKS:
                    nc.sync.dma_start(xp_t[:, r0:r1, :],
                                      xd.ap()[b, :, r0:r1, :])
                return xp_t

            def store(b, o_img, chunks):
                for m, s0, s1 in chunks:
                    nc.scalar.dma_start(
                        yd.ap()[b, m * 128:(m + 1) * 128, s0:s1, :],
                        o_img[m][:, s0 * W:s1 * W])

            def body():
                for b in range(B_CORE):
                    tiles = TILES_TAIL if b == B_CORE - 1 else TILES_8
                    chunks = (STORE_CHUNKS_TAIL if b == B_CORE - 1
                              else STORE_CHUNKS_8)
                    last = (LAST_CHUNK_TAIL if b == B_CORE - 1
                            else LAST_CHUNK_8)
                    xp_t = load_x(b)
                    o_img = {}
                    if probe != "peonly":
                        for m in range(2):
                            o_img[m] = opool.tile([128, H * W], BF16,
                                                  name=f"o_img_{b}_{m}",
                                                  tag="o_img")
                    for t, (r0, nr) in enumerate(tiles):
                        for m in range(2):
                            ps = ppool.tile([128, nr * W], F32)
                            kks = KKS[m]
                            for i, kk in enumerate(kks):
                                kh, kw = divmod(kk, 3)
                                base, msize = WIN[(m, kk)]
                                col = WCOL[(m, kk)]
                                rhs = xp_t[:, r0 + kh: r0 + kh + nr,
                                           kw: kw + W]
                                nc.tensor.matmul(
                                    ps[base:base + msize, :],
                                    w_t[:, col:col + msize], rhs,
                                    start=(i == 0), stop=(i == len(kks) - 1))
                            if probe == "peonly":
                                continue
                            # split PSUM evacuation across ACT and DVE; whole
                            # image-half accumulates in SBUF so stores are a
                            # few big descriptors per partition
                            osl = o_img[m][:, r0 * W:(r0 + nr) * W]
                            if m == 0:
                                nc.scalar.add(osl, ps[:], bias_t[:, m:m + 1])
                                if t in chunks:
                                    store(b, o_img, chunks[t])
                            else:
                                nc.vector.tensor_scalar_add(
                                    osl, ps[:], bias_t[:, m:m + 1])
                    if probe != "peonly":
                        store(b, o_img, last)

            # Warmup matmuls on a scratch tile during the DMA-load head: the
            # PE HAM activity window starts seeing a busy PE at t~0, so the
            # 1.2->2.4 GHz un-throttle fires ~1-2us earlier than if the first
            # real matmul (gated on the x DMA) started the clock.  A few
            # large-free warmups extend continuous PE busy across the ~2.5us
            # DMA head so the ramp never resets before the real stream.
            # split memset so the first (64-wide) warmups aren't gated on
            # zeroing the whole 448-wide scratch: DVE clears the first 64
            # cols (~70ns), GPSIMD the rest in parallel
            wm_src = wpool.tile([C_IN, 448], BF16, name="wm_src")
            nc.vector.memset(wm_src[:, :64], 0.0)
            nc.gpsimd.memset(wm_src[:, 64:], 0.0)
            wm_ps = ppool.tile([64, FREE], F32, name="wm_ps", tag="ps")
            for _ in range(6):
                nc.tensor.matmul(wm_ps[:, :64], wm_src[:, :64],
                                 wm_src[:, :64], start=True, stop=True)
            for _ in range(6):
                nc.tensor.matmul(wm_ps[:], wm_src[:, :64], wm_src[:],
                                 start=True, stop=True)

            if repeat == 1:
                body()
            else:
                with tc.For_i(0, repeat, 1,
                              hint_engines=(mybir.EngineType.PE,
                                            mybir.EngineType.Activation,
                                            mybir.EngineType.SP)):
                    body()
    nc.finalize()
    return nc


def _prep_inputs(x: np.ndarray, twiddle: np.ndarray, bias: np.ndarray):
    """Host-side: pad + cast x, compose weights, arrange per-core in_maps."""
    x = np.asarray(x, dtype=np.float32)
    xpad = np.zeros((B, C_IN, HP, WP), dtype=ml_dtypes.bfloat16)
    xpad[:, :, 1:1 + H, 1:1 + W] = x.astype(ml_dtypes.bfloat16)

    M = _debut_matrix(np.asarray(twiddle, dtype=np.float32))
    wmat = np.zeros((C_IN, W_COLS), dtype=np.float64)
    for (m, kk) in PAIRS:
        # lhsT layout: wmat[c, WCOL + i] = M[128m + base + i, 128kk + c]
        base, msize = WIN[(m, kk)]
        col = WCOL[(m, kk)]
        wmat[:, col:col + msize] = M[m * 128 + base:m * 128 + base + msize,
                                     kk * 128:(kk + 1) * 128].T
    wmat = wmat.astype(ml_dtypes.bfloat16)

    bias2 = np.asarray(bias, dtype=np.float32).reshape(2, 128).T.copy()

    in_maps = []
    for core in range(N_CORES):
        in_maps.append({
            "xpad": xpad[core * B_CORE:(core + 1) * B_CORE],
            "wmat": wmat,
            "bias2": bias2,
        })
    return in_maps


def kernel(x: np.ndarray, twiddle: np.ndarray, bias: np.ndarray) -> np.ndarray:
    if "nc" not in _CACHE:
        _CACHE["nc"] = _build_nc()
    nc = _CACHE["nc"]
    in_maps = _prep_inputs(x, twiddle, bias)
    res = run_bass_kernel_spmd(nc, in_maps, list(range(N_CORES)))
    out = np.concatenate(
        [np.asarray(res.results[i]["y"]) for i in range(N_CORES)], axis=0)
    return np.ascontiguousarray(out.astype(np.float32))
